# revision 6
# baseline (speedup 1.0000x reference)
"""GNN message-passing layer on 8 Trainium2 NeuronCores.

Math (per batch b, sharded one batch per core):
    mask[j,i]   = (adjacency[j,i] > 0)                      [N,N]
    node_part   = ne @ W_node                               [N,H]
    msg_nodes   = mask @ node_part                          [N,H]
    agg_edge[j] = sum_i mask[j,i] * er[i,j,:]               [N,3]
    deg[j]      = sum_i mask[j,i]
    messages    = msg_nodes + agg_edge @ W_edge + deg*b_msg
    out         = relu(ne @ Wu_n + messages @ Wu_m + b_upd)

Folded form actually computed (associativity of the linear maps):
    W_nm  = W_node @ Wu_m         np2' = ne @ W_nm + b_mm   (b_mm = b_msg @ Wu_m)
    W_em  = W_edge @ Wu_m
    out   = relu( neT.T @ Wu_n  +  maskT.T @ np2'  +  agg_edge @ W_em + b_upd )
(the deg*b_mm term falls out of maskT.T @ np2' since sum_i mask[j,i]*b_mm = deg[j]*b_mm)

agg_edge is computed on the PE with a diagonal-extraction trick: for each
128-wide j-block, G[j,(j',e)] = sum_i maskT[i,j]*er[i,j',e] (n=384 matmul);
the wanted values are the j==j' "diagonal", selected with a 0/1 mask S and
a strided free-dim reduce on the vector engine.
"""

import numpy as np

import concourse.bacc as bacc
import concourse.mybir as mybir
import concourse.tile as tile
from concourse import bass_utils

B, N, H = 8, 1024, 256
NCORES = 8
P = 128
NIC = N // P          # 8 i (source-node) chunks
NJB = N // P          # 8 j (dest-node) blocks
EW = 3 * P            # 384 = per-j-block width of the edge slab
F32 = mybir.dt.float32

_CACHE = {}


def _emit(ctx, tc, nc, ne_d, er_d, adj_d, wmsg_d, bmsg_d, wupd_d, bupd_d,
          ident_d, smask_d, out_d, wem_s, bmm_s):
    add = mybir.AluOpType.add
    mult = mybir.AluOpType.mult

    consts = ctx.enter_context(tc.tile_pool(name="consts", bufs=1))
    tp = ctx.enter_context(tc.tile_pool(name="tp_psum", bufs=2, space="PSUM"))
    wp = ctx.enter_context(tc.tile_pool(name="wp_psum", bufs=2, space="PSUM"))

    # ---- constants / weights --------------------------------------------
    ident_t = consts.tile([P, P], F32)
    nc.sync.dma_start(ident_t[:], ident_d.ap())
    smask_t = consts.tile([P, EW], F32)
    nc.sync.dma_start(smask_t[:], smask_d.ap())
    bupd_b = consts.tile([P, H], F32)
    nc.sync.dma_start(
        bupd_b[:], bupd_d.ap()[None, :].broadcast_to([P, H]))

    wu_n, wu_m, wn = [], [], []
    for hc in range(2):
        t = consts.tile([P, H], F32, tag=f"wu_n{hc}")
        nc.sync.dma_start(t[:], wupd_d.ap()[hc * P:(hc + 1) * P, :])
        wu_n.append(t)
        t = consts.tile([P, H], F32, tag=f"wu_m{hc}")
        nc.sync.dma_start(t[:], wupd_d.ap()[H + hc * P:H + (hc + 1) * P, :])
        wu_m.append(t)
        t = consts.tile([P, H], F32, tag=f"wn{hc}")
        nc.sync.dma_start(t[:], wmsg_d.ap()[hc * P:(hc + 1) * P, :])
        wn.append(t)
    wedge_t = consts.tile([3, H], F32)
    nc.sync.dma_start(wedge_t[:], wmsg_d.ap()[2 * P:2 * P + 3, :])
    bmsg_col = []
    for hc in range(2):
        t = consts.tile([P, 1], F32, tag=f"bmsg{hc}")
        nc.sync.dma_start(
            t[:], bmsg_d.ap()[hc * P:(hc + 1) * P][:, None])
        bmsg_col.append(t)

    # ---- node embeddings + transpose ------------------------------------
    ne_t = consts.tile([P, NIC * H], F32)       # [p, (ic h)]
    nc.sync.dma_start(
        ne_t[:].rearrange("p (c h) -> p c h", h=H),
        ne_d.ap().rearrange("(c p) h -> p c h", p=P))
    neT = [consts.tile([P, N], F32, name=f"neT{hc}", tag=f"neT{hc}")
           for hc in range(2)]
    for ic in range(NIC):
        for hc in range(2):
            pt = tp.tile([P, P], F32)
            nc.tensor.transpose(
                pt[:], ne_t[:, ic * H + hc * P:ic * H + (hc + 1) * P], ident_t[:])
            nc.vector.tensor_copy(neT[hc][:, ic * P:(ic + 1) * P], pt[:])

    # ---- adjacency -> f32 mask -> transposed mask ------------------------
    adj_pool = ctx.enter_context(tc.tile_pool(name="adj", bufs=2))
    mask_pool = ctx.enter_context(tc.tile_pool(name="mask", bufs=2))
    maskT = [consts.tile([P, N], F32, name=f"maskT{ic}", tag=f"maskT{ic}")
             for ic in range(NIC)]
    for jc in range(NJB):
        adj_t = adj_pool.tile([P, N], mybir.dt.int32)
        nc.sync.dma_start(adj_t[:], adj_d.ap()[jc * P:(jc + 1) * P, :])
        mask_t = mask_pool.tile([P, N], F32)
        nc.gpsimd.tensor_scalar(mask_t[:], adj_t[:], 0, None,
                                op0=mybir.AluOpType.is_gt)
        for ic in range(NIC):
            pt = tp.tile([P, P], F32)
            nc.tensor.transpose(
                pt[:], mask_t[:, ic * P:(ic + 1) * P], ident_t[:])
            nc.vector.tensor_copy(maskT[ic][:, jc * P:(jc + 1) * P], pt[:])

    # ---- folded weights on-device ---------------------------------------
    # W_nodeT (for W_nm = W_node @ Wu_m)
    wnT = [consts.tile([P, H], F32, name=f"wnT{h2c}", tag=f"wnT{h2c}")
           for h2c in range(2)]
    for hc in range(2):
        for h2c in range(2):
            pt = tp.tile([P, P], F32)
            nc.tensor.transpose(
                pt[:], wn[hc][:, h2c * P:(h2c + 1) * P], ident_t[:])
            nc.vector.tensor_copy(wnT[h2c][:, hc * P:(hc + 1) * P], pt[:])
    w_nm = [consts.tile([P, H], F32, name=f"w_nm{hc}", tag=f"w_nm{hc}")
            for hc in range(2)]
    for hc in range(2):
        pw = wp.tile([P, H], F32, tag="wp")
        for h2c in range(2):
            nc.tensor.matmul(pw[:], wnT[h2c][:, hc * P:(hc + 1) * P],
                             wu_m[h2c][:], start=(h2c == 0), stop=(h2c == 1))
        nc.vector.tensor_copy(w_nm[hc][:], pw[:])

    # W_em = W_edge @ Wu_m, broadcast across partitions via DRAM roundtrip
    wedgeT = [consts.tile([P, 3], F32, name=f"wedgeT{h2c}", tag=f"wedgeT{h2c}")
              for h2c in range(2)]
    for h2c in range(2):
        pt = tp.tile([P, P], F32)
        nc.tensor.transpose(
            pt[:, 0:3], wedge_t[:, h2c * P:(h2c + 1) * P], ident_t[0:3, 0:3])
        nc.vector.tensor_copy(wedgeT[h2c][:], pt[:, 0:3])
    pw = wp.tile([P, H], F32, tag="wp")
    for h2c in range(2):
        nc.tensor.matmul(pw[0:3, :], wedgeT[h2c][:], wu_m[h2c][:],
                         start=(h2c == 0), stop=(h2c == 1))
    wem_sb = consts.tile([3, H], F32)
    nc.vector.tensor_copy(wem_sb[:], pw[0:3, :])
    nc.sync.dma_start(wem_s.ap(), wem_sb[:])
    wem_b = consts.tile([P, 3 * H], F32)
    for e in range(3):
        nc.sync.dma_start(wem_b[:, e * H:(e + 1) * H],
                          wem_s.ap()[e:e + 1, :].broadcast_to([P, H]))

    # b_mm = b_msg @ Wu_m, broadcast
    pb = wp.tile([P, H], F32, tag="wp")
    for h2c in range(2):
        nc.tensor.matmul(pb[0:1, :], bmsg_col[h2c][:], wu_m[h2c][:],
                         start=(h2c == 0), stop=(h2c == 1))
    bmm_sb = consts.tile([1, H], F32)
    nc.vector.tensor_copy(bmm_sb[:], pb[0:1, :])
    nc.sync.dma_start(bmm_s.ap(), bmm_sb[:])
    bmm_b = consts.tile([P, H], F32)
    nc.sync.dma_start(bmm_b[:], bmm_s.ap()[0:1, :].broadcast_to([P, H]))

    # np2' = ne @ W_nm + b_mm
    np2 = consts.tile([P, NIC * H], F32)
    for ic in range(NIC):
        pn = wp.tile([P, H], F32, tag="wp")
        for hc in range(2):
            nc.tensor.matmul(pn[:], neT[hc][:, ic * P:(ic + 1) * P],
                             w_nm[hc][:], start=(hc == 0), stop=(hc == 1))
        nc.vector.tensor_add(np2[:, ic * H:(ic + 1) * H], pn[:], bmm_b[:])

    # ---- main loop over destination j-blocks ----------------------------
    er_pool = ctx.enter_context(tc.tile_pool(name="er", bufs=2))
    pg_pool = ctx.enter_context(tc.tile_pool(name="pg", bufs=2, space="PSUM"))
    po_pool = ctx.enter_context(tc.tile_pool(name="po", bufs=2, space="PSUM"))
    tmp_pool = ctx.enter_context(tc.tile_pool(name="tmp", bufs=2))
    out_pool = ctx.enter_context(tc.tile_pool(name="outp", bufs=2))

    er3 = er_d.ap().rearrange("(c p) j e -> p c (j e)", p=P)  # [P, NIC, N*3]
    for jb in range(NJB):
        er_t = er_pool.tile([P, NIC * EW], F32)
        nc.sync.dma_start(
            er_t[:].rearrange("p (c w) -> p c w", w=EW),
            er3[:, :, jb * EW:(jb + 1) * EW])
        pg = pg_pool.tile([P, EW], F32)
        for ic in range(NIC):
            nc.tensor.matmul(pg[:], maskT[ic][:, jb * P:(jb + 1) * P],
                             er_t[:, ic * EW:(ic + 1) * EW],
                             start=(ic == 0), stop=(ic == NIC - 1))
        po = po_pool.tile([P, H], F32)
        for ic in range(NIC):
            nc.tensor.matmul(po[:], maskT[ic][:, jb * P:(jb + 1) * P],
                             np2[:, ic * H:(ic + 1) * H],
                             start=(ic == 0), stop=False)
        for hc in range(2):
            nc.tensor.matmul(po[:], neT[hc][:, jb * P:(jb + 1) * P],
                             wu_n[hc][:], start=False, stop=(hc == 1))

        # diagonal extraction: agg[j, e] = sum_j' S[j,(j',e)] * G[j,(j',e)]
        tmp_d = tmp_pool.tile([P, EW], F32)
        nc.vector.tensor_mul(tmp_d[:], pg[:], smask_t[:])
        agg = tmp_pool.tile([P, 4], F32)
        nc.vector.tensor_reduce(
            agg[:, 0:3], tmp_d[:].rearrange("p (j e) -> p e j", e=3),
            axis=mybir.AxisListType.X, op=add)

        # edge/bias chain + relu
        t0 = tmp_pool.tile([P, H], F32, tag="t0")
        nc.vector.scalar_tensor_tensor(t0[:], wem_b[:, 0:H], agg[:, 0:1],
                                       bupd_b[:], op0=mult, op1=add)
        t1 = tmp_pool.tile([P, H], F32, tag="t1")
        nc.vector.scalar_tensor_tensor(t1[:], wem_b[:, H:2 * H], agg[:, 1:2],
                                       t0[:], op0=mult, op1=add)
        t2 = tmp_pool.tile([P, H], F32, tag="t2")
        nc.vector.scalar_tensor_tensor(t2[:], wem_b[:, 2 * H:3 * H],
                                       agg[:, 2:3], t1[:], op0=mult, op1=add)
        t3 = tmp_pool.tile([P, H], F32, tag="t3")
        nc.vector.scalar_tensor_tensor(t3[:], po[:], 1.0, t2[:],
                                       op0=mult, op1=add)
        out_t = out_pool.tile([P, H], F32)
        nc.vector.tensor_scalar(out_t[:], t3[:], 0.0, None,
                                op0=mybir.AluOpType.max)
        nc.sync.dma_start(out_d.ap()[jb * P:(jb + 1) * P, :], out_t[:])


def _build():
    if "nc" in _CACHE:
        return _CACHE["nc"]
    from contextlib import ExitStack

    nc = bacc.Bacc("TRN2", target_bir_lowering=False, debug=False,
                   num_devices=NCORES)
    ne_d = nc.dram_tensor("ne", [N, H], F32, kind="ExternalInput")
    er_d = nc.dram_tensor("er", [N, N, 3], F32, kind="ExternalInput")
    adj_d = nc.dram_tensor("adj", [N, N], mybir.dt.int32, kind="ExternalInput")
    wmsg_d = nc.dram_tensor("wmsg", [H + 3, H], F32, kind="ExternalInput")
    bmsg_d = nc.dram_tensor("bmsg", [H], F32, kind="ExternalInput")
    wupd_d = nc.dram_tensor("wupd", [2 * H, H], F32, kind="ExternalInput")
    bupd_d = nc.dram_tensor("bupd", [H], F32, kind="ExternalInput")
    out_d = nc.dram_tensor("out", [N, H], F32, kind="ExternalOutput")
    wem_s = nc.dram_tensor("wem_scratch", [3, H], F32)
    bmm_s = nc.dram_tensor("bmm_scratch", [1, H], F32)

    ident_np = np.eye(P, dtype=np.float32)
    smask_np = np.zeros((P, EW), dtype=np.float32)
    for j in range(P):
        smask_np[j, 3 * j:3 * j + 3] = 1.0
    ident_d = nc.inline_tensor(ident_np, name="ident")
    smask_d = nc.inline_tensor(smask_np, name="smask")

    with tile.TileContext(nc) as tc:
        with ExitStack() as ctx:
            _emit(ctx, tc, nc, ne_d, er_d, adj_d, wmsg_d, bmsg_d, wupd_d,
                  bupd_d, ident_d, smask_d, out_d, wem_s, bmm_s)
    nc.compile()
    _CACHE["nc"] = nc
    return nc


def kernel(node_embeddings, edge_relations, adjacency, W_msg, b_msg,
           W_upd, b_upd):
    node_embeddings = np.ascontiguousarray(
        np.asarray(node_embeddings, dtype=np.float32))
    edge_relations = np.ascontiguousarray(
        np.asarray(edge_relations, dtype=np.float32))
    adjacency = np.ascontiguousarray(np.asarray(adjacency, dtype=np.int32))
    W_msg = np.ascontiguousarray(np.asarray(W_msg, dtype=np.float32))
    b_msg = np.ascontiguousarray(np.asarray(b_msg, dtype=np.float32))
    W_upd = np.ascontiguousarray(np.asarray(W_upd, dtype=np.float32))
    b_upd = np.ascontiguousarray(np.asarray(b_upd, dtype=np.float32))

    nc = _build()
    in_maps = []
    for b in range(NCORES):
        in_maps.append({
            "ne": np.ascontiguousarray(node_embeddings[b]),
            "er": np.ascontiguousarray(edge_relations[b]),
            "adj": adjacency,
            "wmsg": W_msg,
            "bmsg": b_msg,
            "wupd": W_upd,
            "bupd": b_upd,
        })
    res = bass_utils.run_bass_kernel_spmd(nc, in_maps,
                                          core_ids=list(range(NCORES)))
    return np.stack([res.results[c]["out"] for c in range(NCORES)], axis=0)


# revision 8
# speedup vs baseline: 1.9666x; 1.9666x over previous
"""GNN message-passing layer on 8 Trainium2 NeuronCores.

Math (per batch b, sharded one batch per core):
    mask[j,i]   = (adjacency[j,i] > 0)                      [N,N]
    node_part   = ne @ W_node                               [N,H]
    msg_nodes   = mask @ node_part                          [N,H]
    agg_edge[j] = sum_i mask[j,i] * er[i,j,:]               [N,3]
    deg[j]      = sum_i mask[j,i]
    messages    = msg_nodes + agg_edge @ W_edge + deg*b_msg
    out         = relu(ne @ Wu_n + messages @ Wu_m + b_upd)

Folded form actually computed (associativity of the linear maps):
    W_nm  = W_node @ Wu_m         np2' = ne @ W_nm + b_mm   (b_mm = b_msg @ Wu_m)
    W_em  = W_edge @ Wu_m
    out   = relu( neT.T @ Wu_n  +  maskT.T @ np2'  +  agg_edge @ W_em + b_upd )
(the deg*b_mm term falls out of maskT.T @ np2' since sum_i mask[j,i]*b_mm = deg[j]*b_mm)

agg_edge is computed on the PE with a diagonal-extraction trick: for each
128-wide j-block, G[j,(j',e)] = sum_i maskT[i,j]*er[i,j',e] (n=384 matmul);
the wanted values are the j==j' "diagonal", selected with a 0/1 mask S and
a strided free-dim reduce on the vector engine.
"""

import numpy as np

import concourse.bacc as bacc
import concourse.mybir as mybir
import concourse.tile as tile
from concourse import bass_utils

B, N, H = 8, 1024, 256
NCORES = 8
P = 128
NIC = N // P          # 8 i (source-node) chunks
NJB = N // P          # 8 j (dest-node) blocks
EW = 3 * P            # 384 = per-j-block width of the edge slab
F32 = mybir.dt.float32
F32R = mybir.dt.float32r

_CACHE = {}


def _emit(ctx, tc, nc, ne_d, er_d, adj_d, wmsg_d, bmsg_d, wupd_d, bupd_d,
          ident_d, smask_d, out_d, wem_s, bmm_s):
    add = mybir.AluOpType.add
    mult = mybir.AluOpType.mult

    def mmr(out, lhsT, rhs, **kw):
        # operands are float32r tiles: single-pass fp32 matmul (4x vs LOW_HIGH)
        nc.tensor.matmul(out, lhsT, rhs, **kw)

    consts = ctx.enter_context(tc.tile_pool(name="consts", bufs=1))
    tp = ctx.enter_context(tc.tile_pool(name="tp_psum", bufs=2, space="PSUM"))
    wp = ctx.enter_context(tc.tile_pool(name="wp_psum", bufs=2, space="PSUM"))

    # ---- constants / weights --------------------------------------------
    ident_t = consts.tile([P, P], F32)
    nc.sync.dma_start(ident_t[:], ident_d.ap())
    smask_t = consts.tile([P, EW], F32)
    nc.sync.dma_start(smask_t[:], smask_d.ap())
    bupd_b = consts.tile([P, H], F32)
    nc.sync.dma_start(
        bupd_b[:], bupd_d.ap()[None, :].broadcast_to([P, H]))

    wu_n, wu_m, wn = [], [], []
    for hc in range(2):
        t = consts.tile([P, H], F32R, tag=f"wu_n{hc}")
        nc.sync.dma_start(t[:], wupd_d.ap()[hc * P:(hc + 1) * P, :])
        wu_n.append(t)
        t = consts.tile([P, H], F32R, tag=f"wu_m{hc}")
        nc.sync.dma_start(t[:], wupd_d.ap()[H + hc * P:H + (hc + 1) * P, :])
        wu_m.append(t)
        t = consts.tile([P, H], F32, tag=f"wn{hc}")
        nc.sync.dma_start(t[:], wmsg_d.ap()[hc * P:(hc + 1) * P, :])
        wn.append(t)
    wedge_t = consts.tile([3, H], F32)
    nc.sync.dma_start(wedge_t[:], wmsg_d.ap()[2 * P:2 * P + 3, :])
    bmsg_col = []
    for hc in range(2):
        t = consts.tile([P, 1], F32R, tag=f"bmsg{hc}")
        nc.sync.dma_start(
            t[:], bmsg_d.ap()[hc * P:(hc + 1) * P][:, None])
        bmsg_col.append(t)

    # ---- node embeddings + transpose ------------------------------------
    ne_t = consts.tile([P, NIC * H], F32)       # [p, (ic h)]
    nc.sync.dma_start(
        ne_t[:].rearrange("p (c h) -> p c h", h=H),
        ne_d.ap().rearrange("(c p) h -> p c h", p=P))
    neT = [consts.tile([P, N], F32R, name=f"neT{hc}", tag=f"neT{hc}")
           for hc in range(2)]
    for hc in range(2):
        for q in range(2):
            pt = tp.tile([P, 4 * P], F32)
            for k in range(4):
                ic = q * 4 + k
                nc.tensor.transpose(
                    pt[:, k * P:(k + 1) * P],
                    ne_t[:, ic * H + hc * P:ic * H + (hc + 1) * P], ident_t[:])
            nc.vector.tensor_copy(
                neT[hc][:, q * 4 * P:(q + 1) * 4 * P], pt[:])

    # ---- adjacency -> f32 mask -> transposed mask ------------------------
    adj_pool = ctx.enter_context(tc.tile_pool(name="adj", bufs=2))
    mask_pool = ctx.enter_context(tc.tile_pool(name="mask", bufs=1))
    maskT = [consts.tile([P, N], F32R, name=f"maskT{ic}", tag=f"maskT{ic}")
             for ic in range(NIC)]
    masks = []
    for jc in range(NJB):
        adj_t = adj_pool.tile([P, N], mybir.dt.int32)
        nc.sync.dma_start(adj_t[:], adj_d.ap()[jc * P:(jc + 1) * P, :])
        mask_t = mask_pool.tile([P, N], F32, name=f"mask{jc}", tag=f"mask{jc}")
        nc.vector.tensor_scalar(mask_t[:], adj_t[:], 0, None,
                                op0=mybir.AluOpType.is_gt)
        masks.append(mask_t)
    for ic in range(NIC):
        for q in range(2):
            pt = tp.tile([P, 4 * P], F32)
            for k in range(4):
                jc = q * 4 + k
                nc.tensor.transpose(
                    pt[:, k * P:(k + 1) * P],
                    masks[jc][:, ic * P:(ic + 1) * P], ident_t[:])
            nc.vector.tensor_copy(
                maskT[ic][:, q * 4 * P:(q + 1) * 4 * P], pt[:])

    # ---- folded weights on-device ---------------------------------------
    # W_nodeT (for W_nm = W_node @ Wu_m)
    wnT = [consts.tile([P, H], F32R, name=f"wnT{h2c}", tag=f"wnT{h2c}")
           for h2c in range(2)]
    for h2c in range(2):
        pt = tp.tile([P, 2 * P], F32)
        for hc in range(2):
            nc.tensor.transpose(
                pt[:, hc * P:(hc + 1) * P],
                wn[hc][:, h2c * P:(h2c + 1) * P], ident_t[:])
        nc.vector.tensor_copy(wnT[h2c][:], pt[:])
    w_nm = [consts.tile([P, H], F32R, name=f"w_nm{hc}", tag=f"w_nm{hc}")
            for hc in range(2)]
    for hc in range(2):
        pw = wp.tile([P, H], F32, tag="wp")
        for h2c in range(2):
            mmr(pw[:], wnT[h2c][:, hc * P:(hc + 1) * P],
                wu_m[h2c][:], start=(h2c == 0), stop=(h2c == 1))
        nc.vector.tensor_copy(w_nm[hc][:], pw[:])

    # W_em = W_edge @ Wu_m, broadcast across partitions via DRAM roundtrip
    wedgeT = [consts.tile([P, 3], F32R, name=f"wedgeT{h2c}", tag=f"wedgeT{h2c}")
              for h2c in range(2)]
    for h2c in range(2):
        pt = tp.tile([P, P], F32)
        nc.tensor.transpose(
            pt[:, 0:3], wedge_t[:, h2c * P:(h2c + 1) * P], ident_t[0:3, 0:3])
        nc.vector.tensor_copy(wedgeT[h2c][:], pt[:, 0:3])
    pw = wp.tile([P, H], F32, tag="wp")
    for h2c in range(2):
        mmr(pw[0:3, :], wedgeT[h2c][:], wu_m[h2c][:],
            start=(h2c == 0), stop=(h2c == 1))
    wem_sb = consts.tile([3, H], F32)
    nc.vector.tensor_copy(wem_sb[:], pw[0:3, :])
    nc.sync.dma_start(wem_s.ap(), wem_sb[:])
    wem_b = consts.tile([P, 3 * H], F32)
    for e in range(3):
        nc.sync.dma_start(wem_b[:, e * H:(e + 1) * H],
                          wem_s.ap()[e:e + 1, :].broadcast_to([P, H]))

    # b_mm = b_msg @ Wu_m, broadcast
    pb = wp.tile([P, H], F32, tag="wp")
    for h2c in range(2):
        mmr(pb[0:1, :], bmsg_col[h2c][:], wu_m[h2c][:],
            start=(h2c == 0), stop=(h2c == 1))
    bmm_sb = consts.tile([1, H], F32)
    nc.vector.tensor_copy(bmm_sb[:], pb[0:1, :])
    nc.sync.dma_start(bmm_s.ap(), bmm_sb[:])
    bmm_b = consts.tile([P, H], F32)
    nc.sync.dma_start(bmm_b[:], bmm_s.ap()[0:1, :].broadcast_to([P, H]))

    # np2' = ne @ W_nm + b_mm
    np2 = consts.tile([P, NIC * H], F32R)
    for ic in range(NIC):
        pn = wp.tile([P, H], F32, tag="wp")
        for hc in range(2):
            mmr(pn[:], neT[hc][:, ic * P:(ic + 1) * P],
                w_nm[hc][:], start=(hc == 0), stop=(hc == 1))
        nc.vector.tensor_add(np2[:, ic * H:(ic + 1) * H], pn[:], bmm_b[:])

    # ---- main loop over destination j-blocks ----------------------------
    er_pool = ctx.enter_context(tc.tile_pool(name="er", bufs=2))
    pg_pool = ctx.enter_context(tc.tile_pool(name="pg", bufs=2, space="PSUM"))
    po_pool = ctx.enter_context(tc.tile_pool(name="po", bufs=2, space="PSUM"))
    tmp_pool = ctx.enter_context(tc.tile_pool(name="tmp", bufs=2))
    out_pool = ctx.enter_context(tc.tile_pool(name="outp", bufs=2))

    er3 = er_d.ap().rearrange("(c p) j e -> p c (j e)", p=P)  # [P, NIC, N*3]
    for jb in range(NJB):
        er_t = er_pool.tile([P, NIC * EW], F32R)
        nc.sync.dma_start(
            er_t[:].rearrange("p (c w) -> p c w", w=EW),
            er3[:, :, jb * EW:(jb + 1) * EW])
        pg = pg_pool.tile([P, EW], F32)
        for ic in range(NIC):
            mmr(pg[:], maskT[ic][:, jb * P:(jb + 1) * P],
                er_t[:, ic * EW:(ic + 1) * EW],
                start=(ic == 0), stop=(ic == NIC - 1))
        po = po_pool.tile([P, H], F32)
        for ic in range(NIC):
            mmr(po[:], maskT[ic][:, jb * P:(jb + 1) * P],
                np2[:, ic * H:(ic + 1) * H],
                start=(ic == 0), stop=False)
        for hc in range(2):
            mmr(po[:], neT[hc][:, jb * P:(jb + 1) * P],
                wu_n[hc][:], start=False, stop=(hc == 1))

        # diagonal extraction: agg[j, e] = sum_j' S[j,(j',e)] * G[j,(j',e)]
        tmp_d = tmp_pool.tile([P, EW], F32)
        nc.vector.tensor_mul(tmp_d[:], pg[:], smask_t[:])
        agg = tmp_pool.tile([P, 4], F32)
        nc.vector.tensor_reduce(
            agg[:, 0:3], tmp_d[:].rearrange("p (j e) -> p e j", e=3),
            axis=mybir.AxisListType.X, op=add)

        # edge/bias chain + relu
        t0 = tmp_pool.tile([P, H], F32, tag="t0")
        nc.vector.scalar_tensor_tensor(t0[:], wem_b[:, 0:H], agg[:, 0:1],
                                       bupd_b[:], op0=mult, op1=add)
        t1 = tmp_pool.tile([P, H], F32, tag="t1")
        nc.vector.scalar_tensor_tensor(t1[:], wem_b[:, H:2 * H], agg[:, 1:2],
                                       t0[:], op0=mult, op1=add)
        t2 = tmp_pool.tile([P, H], F32, tag="t2")
        nc.vector.scalar_tensor_tensor(t2[:], wem_b[:, 2 * H:3 * H],
                                       agg[:, 2:3], t1[:], op0=mult, op1=add)
        t3 = tmp_pool.tile([P, H], F32, tag="t3")
        nc.vector.scalar_tensor_tensor(t3[:], po[:], 1.0, t2[:],
                                       op0=mult, op1=add)
        out_t = out_pool.tile([P, H], F32)
        nc.vector.tensor_scalar(out_t[:], t3[:], 0.0, None,
                                op0=mybir.AluOpType.max)
        nc.sync.dma_start(out_d.ap()[jb * P:(jb + 1) * P, :], out_t[:])


def _build():
    if "nc" in _CACHE:
        return _CACHE["nc"]
    from contextlib import ExitStack

    nc = bacc.Bacc("TRN2", target_bir_lowering=False, debug=False,
                   num_devices=NCORES)
    ne_d = nc.dram_tensor("ne", [N, H], F32, kind="ExternalInput")
    er_d = nc.dram_tensor("er", [N, N, 3], F32R, kind="ExternalInput")
    adj_d = nc.dram_tensor("adj", [N, N], mybir.dt.int32, kind="ExternalInput")
    wmsg_d = nc.dram_tensor("wmsg", [H + 3, H], F32, kind="ExternalInput")
    bmsg_d = nc.dram_tensor("bmsg", [H], F32R, kind="ExternalInput")
    wupd_d = nc.dram_tensor("wupd", [2 * H, H], F32R, kind="ExternalInput")
    bupd_d = nc.dram_tensor("bupd", [H], F32, kind="ExternalInput")
    out_d = nc.dram_tensor("out", [N, H], F32, kind="ExternalOutput")
    wem_s = nc.dram_tensor("wem_scratch", [3, H], F32)
    bmm_s = nc.dram_tensor("bmm_scratch", [1, H], F32)

    ident_np = np.eye(P, dtype=np.float32)
    smask_np = np.zeros((P, EW), dtype=np.float32)
    for j in range(P):
        smask_np[j, 3 * j:3 * j + 3] = 1.0
    ident_d = nc.inline_tensor(ident_np, name="ident")
    smask_d = nc.inline_tensor(smask_np, name="smask")

    with tile.TileContext(nc) as tc:
        with ExitStack() as ctx:
            _emit(ctx, tc, nc, ne_d, er_d, adj_d, wmsg_d, bmsg_d, wupd_d,
                  bupd_d, ident_d, smask_d, out_d, wem_s, bmm_s)
    nc.compile()
    _CACHE["nc"] = nc
    return nc


def kernel(node_embeddings, edge_relations, adjacency, W_msg, b_msg,
           W_upd, b_upd):
    node_embeddings = np.ascontiguousarray(
        np.asarray(node_embeddings, dtype=np.float32))
    edge_relations = np.ascontiguousarray(
        np.asarray(edge_relations, dtype=np.float32))
    adjacency = np.ascontiguousarray(np.asarray(adjacency, dtype=np.int32))
    W_msg = np.ascontiguousarray(np.asarray(W_msg, dtype=np.float32))
    b_msg = np.ascontiguousarray(np.asarray(b_msg, dtype=np.float32))
    W_upd = np.ascontiguousarray(np.asarray(W_upd, dtype=np.float32))
    b_upd = np.ascontiguousarray(np.asarray(b_upd, dtype=np.float32))

    nc = _build()
    in_maps = []
    for b in range(NCORES):
        in_maps.append({
            "ne": np.ascontiguousarray(node_embeddings[b]),
            "er": np.ascontiguousarray(edge_relations[b]),
            "adj": adjacency,
            "wmsg": W_msg,
            "bmsg": b_msg,
            "wupd": W_upd,
            "bupd": b_upd,
        })
    res = bass_utils.run_bass_kernel_spmd(nc, in_maps,
                                          core_ids=list(range(NCORES)))
    return np.stack([res.results[c]["out"] for c in range(NCORES)], axis=0)
